# revision 4
# baseline (speedup 1.0000x reference)
"""Bass/Trainium2 kernel for nn_LoopFallbackEval: y = x + 4096.0 (elementwise).

Full input x: (16384, 4096) f32. Sharded along dim 0 across 8 NeuronCores
(data parallel, 2048 rows each). Per core: load (128, 4096) tiles, add the
constant on the vector engine (fp32 tensor_scalar runs in 2x perf mode),
store back. Memory-bound: 32 MiB in + 32 MiB out per core.
"""

import numpy as np

_M, _N = 16384, 4096
_N_CORES = 8
_ROWS = _M // _N_CORES  # 2048 rows per core
_P = 128  # SBUF partitions
_N_TILES = _ROWS // _P  # 16 tiles per core

_ADD_CONST = float(_N)  # reference adds x.shape[1] = 4096

_compiled_nc = None


def _build_nc(reps: int = 1):
    import concourse.bacc as bacc
    import concourse.mybir as mybir
    from concourse.tile import TileContext

    # Bacc (not raw Bass): its finalize() runs generate_event_semaphores,
    # which splits multi-sem waits — walrus codegen allows only 1 wait/inst.
    nc = bacc.Bacc(None)
    x_in = nc.dram_tensor("x", [_ROWS, _N], mybir.dt.float32, kind="ExternalInput")
    y_out = nc.dram_tensor("y", [_ROWS, _N], mybir.dt.float32, kind="ExternalOutput")

    xv = x_in[:, :].rearrange("(t p) n -> t p n", p=_P)
    yv = y_out[:, :].rearrange("(t p) n -> t p n", p=_P)

    with TileContext(nc) as tc:
        with tc.tile_pool(name="io", bufs=4) as pool:
            for _ in range(reps):  # reps>1 only for benchmarking (slope method)
                for i in range(_N_TILES):
                    t = pool.tile([_P, _N], mybir.dt.float32)
                    nc.sync.dma_start(out=t[:], in_=xv[i])
                    nc.vector.tensor_scalar_add(t[:], t[:], _ADD_CONST)
                    nc.sync.dma_start(out=yv[i], in_=t[:])
    nc.finalize()
    return nc


def _get_nc():
    global _compiled_nc
    if _compiled_nc is None:
        _compiled_nc = _build_nc()
    return _compiled_nc


def _shard(x: np.ndarray) -> list[dict[str, np.ndarray]]:
    return [
        {"x": np.ascontiguousarray(x[i * _ROWS : (i + 1) * _ROWS])}
        for i in range(_N_CORES)
    ]


def _run(x: np.ndarray, **spmd_kwargs):
    from concourse.bass_utils import run_bass_kernel_spmd

    res = run_bass_kernel_spmd(
        _get_nc(), _shard(x), core_ids=list(range(_N_CORES)), **spmd_kwargs
    )
    out = np.concatenate([r["y"] for r in res.results], axis=0)
    return out, res


def kernel(**inputs: np.ndarray) -> np.ndarray:
    x = np.asarray(inputs["x"], dtype=np.float32)
    assert x.shape == (_M, _N), x.shape
    out, _ = _run(x)
    return out
